# revision 1
# baseline (speedup 1.0000x reference)
"""MiniRocket-style dilated conv features on Trainium2 (Bass/Tile).

Problem: x[16,12,5000] f32, per-dilation ternary weight banks
weights[10,1000,12,9], biases[10,1000].  For each dilation d in
[1,2,...,512]: y = conv1d(x, W_d, rhs_dilation=d, SAME) -> [B,1000,5000];
features are max over time and PPV (mean of y > bias) -> [16, 20000].

Strategy (8 NeuronCores, data-parallel over batch, 2 batches/core):
  - Build a 108-row shifted-input stack Xs[(j,c), t] = x[c, t+(j-4)d]
    (zero padded) in SBUF via DMA, fp16.
  - Conv as TensorE matmuls: out[k, t] = sum_r W^T[r, k] * Xs[r, t],
    contract dim 108, M=125 kernels/tile, N=512 cols/matmul -> fp32 PSUM.
  - Reductions straight off PSUM:
      * ScalarE evicts most chunks PSUM f32 -> SBUF fp16 (ACTIVATE Copy).
      * VectorE tensor_scalar(+accum) does max-reduce at 4x on the fp16
        copies; the tail chunk is evicted+max-reduced by VectorE itself
        (fused, 1x from PSUM) to balance engine load.
      * PPV via tensor_scalar(is_gt bias, accum add) at 4x on fp16.
  - Tiny final merges (reduce over 3 chunk slots) + DMA out.

Host-side prep is layout only: fp16 casts and the W -> W^T[(j,c),k]
transpose.
"""

import numpy as np

import concourse.bacc as bacc
import concourse.bass as bass
import concourse.mybir as mybir
import concourse.tile as tile
from concourse.bass_utils import run_bass_kernel_spmd

import os

ACT_OUT = os.environ.get("MR_ACT_OUT", "sbuf")  # psum | sbuf
DUP = os.environ.get("MR_DUP", "1") == "1"  # dual PSUM copies vs shared
PROBE = os.environ.get("MR_PROBE", "none")  # none | actconst | dveconst

L = 5000
C = 12
KLEN = 9
DILS = [1, 2, 4, 8, 16, 32, 64, 128, 256, 512]
ND = len(DILS)
KPD = 1000
NKT = 8          # kernel tiles per dilation
MT = 125         # kernels per tile (psum partition dim)
NB = 2           # batches per core
NCORES = 8
CONTRACT = C * KLEN  # 108
MM_N = 512
CHUNKS = [(0, 1024), (1024, 2048), (2048, 3072), (3072, 4096), (4096, 5000)]
NCH = len(CHUNKS)
FP16 = mybir.dt.float16
F32 = mybir.dt.float32
ALU = mybir.AluOpType


def _emit(nc, repeat=1, ablate="full"):
    xh = nc.dram_tensor("xh", [NB, C, L], FP16, kind="ExternalInput")
    wt = nc.dram_tensor("wt", [ND, CONTRACT, KPD], FP16, kind="ExternalInput")
    bia = nc.dram_tensor("bia", [MT, ND * NKT], F32, kind="ExternalInput")
    zer = nc.dram_tensor("zer", [C, 2048], FP16, kind="ExternalInput")
    out = nc.dram_tensor("out", [NB, 2 * ND * KPD], F32, kind="ExternalOutput")

    for _rep in range(repeat):
        _emit_body(nc, xh, wt, bia, zer, out, ablate)


def _emit_body(nc, xh, wt, bia, zer, out, ablate="full"):
    do_act = ablate in ("full", "pe_act")
    do_dve = ablate in ("full", "pe_dve")
    with tile.TileContext(nc) as tc:
        with (
            tc.tile_pool(name="const", bufs=1) as constp,
            tc.tile_pool(name="xtp", bufs=2) as xtp,
            tc.tile_pool(name="pspa", bufs=(4 if not DUP else 2), space="PSUM") as pspa,
            tc.tile_pool(name="pspb", bufs=2, space="PSUM") as pspb,
            tc.tile_pool(name="pcp", bufs=1, space="PSUM") as pcp,
            tc.tile_pool(name="finp", bufs=1) as finp,
        ):
            lhsT = constp.tile([CONTRACT, ND * KPD], FP16)
            nc.sync.dma_start(
                lhsT.rearrange("r (d m) -> r d m", d=ND),
                wt.ap().rearrange("d r m -> r d m"),
            )
            negb = constp.tile([MT, ND * NKT], F32)
            nc.sync.dma_start(negb[:, :], bia.ap())
            pconst = None
            if PROBE != "none":
                pconst = pcp.tile([MT, 1024], F32)
                nc.vector.memset(pconst[:, :], 1.0)
            trash_v = constp.tile([MT, 1024], FP16)
            trash_a = constp.tile([MT, 1024], FP16)
            slots_mx = [
                constp.tile([MT, ND * NKT * NCH], F32, name=f"smx{b}")
                for b in range(NB)
            ]
            slots_sg = [
                constp.tile([MT, ND * NKT * NCH], F32, name=f"ssg{b}")
                for b in range(NB)
            ]

            for di, d in enumerate(DILS):
                xt = xtp.tile([CONTRACT, NB * L], FP16, tag="xt", name=f"xt{di}")
                for b in range(NB):
                    base = b * L
                    # Common region t in [4d, L-4d): all 9 taps valid -> one
                    # 108-partition DMA (src taps overlap; reads only).
                    wc = L - 8 * d
                    src = bass.AP(xh, b * C * L + 0, [[d, KLEN], [L, C], [1, wc]])
                    nc.sync.dma_start(xt[:, base + 4 * d : base + 4 * d + wc], src)
                    # Edges + zero padding per tap.
                    for j in range(KLEN):
                        s = (j - 4) * d
                        t0 = max(0, -s)
                        t1 = min(L, L - s)
                        rows = slice(C * j, C * j + C)
                        # valid edge pieces outside the common region
                        for e0, e1 in ((t0, 4 * d), (L - 4 * d, t1)):
                            if e1 > e0:
                                nc.sync.dma_start(
                                    xt[rows, base + e0 : base + e1],
                                    xh.ap()[b, :, e0 + s : e1 + s],
                                )
                        if t0 > 0:
                            nc.sync.dma_start(xt[rows, base : base + t0], zer.ap()[:, 0:t0])
                        if t1 < L:
                            nc.sync.dma_start(
                                xt[rows, base + t1 : base + L], zer.ap()[:, 0 : L - t1]
                            )
                for kt in range(NKT):
                    lhs = lhsT[:, di * KPD + kt * MT : di * KPD + kt * MT + MT]
                    bcol = di * NKT + kt
                    for b in range(NB):
                        for ch, (c0, c1) in enumerate(CHUNKS):
                            w = c1 - c0
                            scol = bcol * NCH + ch
                            # DUP: two independent PSUM copies, A read by
                            # ScalarE, B by VectorE -> no shared-bank
                            # serialization, at the cost of 2x matmuls.
                            # Otherwise one copy read by both (R-R ordered).
                            pa = None
                            if PROBE == "actconst":
                                pass
                            elif do_act or not DUP:
                                pa = pspa.tile(
                                    [MT, 1024], F32, tag="pa",
                                    name=f"pa{di}_{kt}_{b}_{ch}",
                                )
                                for t in range(c0, c1, MM_N):
                                    n = min(MM_N, c1 - t)
                                    nc.tensor.matmul(
                                        pa[:, t - c0 : t - c0 + n],
                                        lhs,
                                        xt[:, b * L + t : b * L + t + n],
                                        start=True,
                                        stop=True,
                                    )
                            if do_act:
                                # ScalarE: PPV via sign(o - b), accumulated sum.
                                # ACT_OUT=psum writes the (unused) sign output
                                # in-place to PSUM (ScE's faster port).
                                pin = pconst if PROBE == "actconst" else pa
                                aout = pin[:, :w] if ACT_OUT == "psum" else trash_a[:, :w]
                                nc.scalar.activation(
                                    aout,
                                    pin[:, :w],
                                    mybir.ActivationFunctionType.Sign,
                                    bias=negb[:, bcol : bcol + 1],
                                    accum_out=slots_sg[b][:, scol : scol + 1],
                                )
                            if do_dve and PROBE == "dveconst":
                                nc.vector.tensor_scalar(
                                    trash_v[:, :w],
                                    pconst[:, :w],
                                    0.0,
                                    None,
                                    op0=ALU.add,
                                    op1=ALU.max,
                                    accum_out=slots_mx[b][:, scol : scol + 1],
                                )
                            elif do_dve:
                                if DUP:
                                    pb = pspb.tile(
                                        [MT, 1024], F32, tag="pb",
                                        name=f"pb{di}_{kt}_{b}_{ch}",
                                    )
                                    for t in range(c0, c1, MM_N):
                                        n = min(MM_N, c1 - t)
                                        nc.tensor.matmul(
                                            pb[:, t - c0 : t - c0 + n],
                                            lhs,
                                            xt[:, b * L + t : b * L + t + n],
                                            start=True,
                                            stop=True,
                                        )
                                else:
                                    pb = pa
                                # VectorE: running max via accum reduce
                                nc.vector.tensor_scalar(
                                    trash_v[:, :w],
                                    pb[:, :w],
                                    0.0,
                                    None,
                                    op0=ALU.add,
                                    op1=ALU.max,
                                    accum_out=slots_mx[b][:, scol : scol + 1],
                                )

            for b in range(NB):
                if not do_act:
                    nc.vector.memset(slots_sg[b][:, :], 0.0)
                if not do_dve:
                    nc.vector.memset(slots_mx[b][:, :], 0.0)

            outv = out.ap().rearrange(
                "bb (d s kt p) -> bb p s d kt", d=ND, s=2, kt=NKT
            )
            for b in range(NB):
                mxr = finp.tile([MT, ND * NKT], F32, name=f"mxr{b}")
                nc.vector.tensor_reduce(
                    mxr[:, :],
                    slots_mx[b].rearrange("p (g c) -> p g c", c=NCH),
                    axis=mybir.AxisListType.X,
                    op=ALU.max,
                )
                sgr = finp.tile([MT, ND * NKT], F32, name=f"sgr{b}")
                nc.vector.tensor_reduce(
                    sgr[:, :],
                    slots_sg[b].rearrange("p (g c) -> p g c", c=NCH),
                    axis=mybir.AxisListType.X,
                    op=ALU.add,
                )
                # ppv = (#gt)/L = (sum_sign + L)/(2L) = sum_sign/(2L) + 0.5
                ppv = finp.tile([MT, ND * NKT], F32, name=f"ppv{b}")
                nc.vector.tensor_scalar(
                    ppv[:, :],
                    sgr[:, :],
                    1.0 / (2.0 * L),
                    0.5,
                    op0=ALU.mult,
                    op1=ALU.add,
                )
                for di in range(ND):
                    nc.sync.dma_start(
                        outv[b, :, 0, di, :], mxr[:, di * NKT : (di + 1) * NKT]
                    )
                    nc.sync.dma_start(
                        outv[b, :, 1, di, :], ppv[:, di * NKT : (di + 1) * NKT]
                    )


_COMPILED = {}


def get_compiled(repeat=1, ablate="full"):
    key = (repeat, ablate)
    if key not in _COMPILED:
        nc = bacc.Bacc(
            "TRN2", target_bir_lowering=False, debug=False, num_devices=NCORES
        )
        _emit(nc, repeat=repeat, ablate=ablate)
        nc.compile()
        _COMPILED[key] = nc
    return _COMPILED[key]


def make_in_maps(x, weights, biases):
    # W[d,k,c,j] -> wt[d, j*12+c, k], matching the Xs row order (j outer, c inner)
    wtr = np.ascontiguousarray(
        weights.astype(np.float16).transpose(0, 3, 2, 1).reshape(ND, CONTRACT, KPD)
    )
    # negated bias, pre-arranged [kernel-in-tile, dilation*ktile] for a
    # contiguous per-partition DMA
    bia = np.ascontiguousarray(
        (-biases.astype(np.float32)).reshape(ND, NKT, MT).transpose(2, 0, 1).reshape(MT, ND * NKT)
    )
    zer = np.zeros((C, 2048), np.float16)
    xh = x.astype(np.float16)
    maps = []
    for c in range(NCORES):
        maps.append(
            {
                "xh": np.ascontiguousarray(xh[NB * c : NB * (c + 1)]),
                "wt": wtr,
                "bia": bia,
                "zer": zer,
            }
        )
    return maps


def run(x, weights, biases, trace=False, **kw):
    nc = get_compiled()
    res = run_bass_kernel_spmd(
        nc, make_in_maps(x, weights, biases), core_ids=list(range(NCORES)),
        trace=trace, **kw
    )
    outs = np.concatenate([r["out"] for r in res.results], axis=0)
    return outs.astype(np.float32), res


def kernel(x, weights, biases):
    out, _ = run(x, weights, biases)
    return out


def bench(x, weights, biases, iters=20, repeat=1, ablate="full"):
    """Time the sharded PJRT executable with pre-staged device inputs.

    Returns (out, per_call_wall_ns_list). Mirrors bass2jax.run_bass_via_pjrt's
    multi-core path, but stages inputs once and times repeated dispatches.
    """
    import time

    import jax
    import jax.numpy as jnp
    from jax.sharding import Mesh, NamedSharding, PartitionSpec
    from jax.experimental.shard_map import shard_map

    import concourse.bass2jax as b2j
    import concourse.mybir as mb

    nc = get_compiled(repeat=repeat, ablate=ablate)
    b2j.install_neuronx_cc_hook()
    in_maps = make_in_maps(x, weights, biases)

    partition_name = nc.partition_id_tensor.name if nc.partition_id_tensor else None
    in_names, out_names, out_avals, zero_outs = [], [], [], []
    for alloc in nc.m.functions[0].allocations:
        if not isinstance(alloc, mb.MemoryLocationSet):
            continue
        name = alloc.memorylocations[0].name
        if alloc.kind == "ExternalInput":
            if name != partition_name:
                in_names.append(name)
        elif alloc.kind == "ExternalOutput":
            out_names.append(name)
            shape = tuple(alloc.tensor_shape)
            dtype = mb.dt.np(alloc.dtype)
            out_avals.append(jax.core.ShapedArray(shape, dtype))
            zero_outs.append(np.zeros(shape, dtype))
    n_params = len(in_names)
    n_outs = len(out_avals)
    all_names = in_names + out_names
    if partition_name is not None:
        all_names = all_names + [partition_name]

    def _body(*args):
        operands = list(args)
        if partition_name is not None:
            operands.append(b2j.partition_id_tensor())
        outs = b2j._bass_exec_p.bind(
            *operands,
            out_avals=tuple(out_avals),
            in_names=tuple(all_names),
            out_names=tuple(out_names),
            lowering_input_output_aliases=(),
            sim_require_finite=True,
            sim_require_nnan=True,
            nc=nc,
        )
        return tuple(outs)

    devices = jax.devices()[:NCORES]
    mesh = Mesh(np.asarray(devices), ("core",))
    spec = PartitionSpec("core")
    sharded = jax.jit(
        shard_map(
            _body,
            mesh=mesh,
            in_specs=(spec,) * (n_params + n_outs),
            out_specs=(spec,) * n_outs,
            check_rep=False,
        ),
        donate_argnums=tuple(range(n_params, n_params + n_outs)),
        keep_unused=True,
    )
    sh = NamedSharding(mesh, spec)
    concat_in = [
        jax.device_put(
            np.concatenate([np.asarray(m[name]) for m in in_maps], axis=0), sh
        )
        for name in in_names
    ]
    zero_host = [np.zeros((NCORES * z.shape[0], *z.shape[1:]), z.dtype) for z in zero_outs]

    times = []
    out_arrs = None
    for i in range(iters + 1):
        zeros_dev = [jax.device_put(z, sh) for z in zero_host]
        jax.block_until_ready(zeros_dev)
        t0 = time.perf_counter()
        out_arrs = sharded(*concat_in, *zeros_dev)
        jax.block_until_ready(out_arrs)
        t1 = time.perf_counter()
        if i > 0:  # skip warmup/compile call
            times.append((t1 - t0) * 1e9)
    out = np.asarray(out_arrs[out_names.index("out")]).reshape(NCORES * NB, -1)
    return out.astype(np.float32), times



# revision 6
# speedup vs baseline: 1.2101x; 1.2101x over previous
"""MiniRocket-style dilated conv features on Trainium2 (Bass/Tile).

Problem: x[16,12,5000] f32, per-dilation ternary weight banks
weights[10,1000,12,9], biases[10,1000].  For each dilation d in
[1,2,...,512]: y = conv1d(x, W_d, rhs_dilation=d, SAME) -> [B,1000,5000];
features are max over time and PPV (mean of y > bias) -> [16, 20000].

Strategy (8 NeuronCores, data-parallel over batch, 2 batches/core):
  - Build a 108-row shifted-input stack Xs[(j,c), t] = x[c, t+(j-4)d]
    (zero padded) in SBUF via DMA, fp16.
  - Conv as TensorE matmuls: out[k, t] = sum_r W^T[r, k] * Xs[r, t],
    contract dim 108, M=125 kernels/tile, N=512 cols/matmul -> fp32 PSUM.
  - Reductions straight off PSUM:
      * ScalarE evicts most chunks PSUM f32 -> SBUF fp16 (ACTIVATE Copy).
      * VectorE tensor_scalar(+accum) does max-reduce at 4x on the fp16
        copies; the tail chunk is evicted+max-reduced by VectorE itself
        (fused, 1x from PSUM) to balance engine load.
      * PPV via tensor_scalar(is_gt bias, accum add) at 4x on fp16.
  - Tiny final merges (reduce over 3 chunk slots) + DMA out.

Host-side prep is layout only: fp16 casts and the W -> W^T[(j,c),k]
transpose.
"""

import numpy as np

import concourse.bacc as bacc
import concourse.bass as bass
import concourse.mybir as mybir
import concourse.tile as tile
from concourse.bass_utils import run_bass_kernel_spmd

import os

ACT_OUT = os.environ.get("MR_ACT_OUT", "sbuf")  # psum | sbuf
DUP = os.environ.get("MR_DUP", "1") == "1"  # dual PSUM copies vs shared
PROBE = os.environ.get("MR_PROBE", "none")  # none | actconst | dveconst

L = 5000
PAD = 2048
LP = PAD + L + PAD  # zero-padded input length
C = 12
KLEN = 9
DILS = [1, 2, 4, 8, 16, 32, 64, 128, 256, 512]
ND = len(DILS)
KPD = 1000
NKT = 8          # kernel tiles per dilation
MT = 125         # kernels per tile (psum partition dim)
NB = 2           # batches per core
NCORES = 8
CONTRACT = C * KLEN  # 108
MM_N = 512
CHUNKS = [(0, 1024), (1024, 2048), (2048, 3072), (3072, 4096), (4096, 5000)]
NCH = len(CHUNKS)
FP16 = mybir.dt.float16
F32 = mybir.dt.float32
ALU = mybir.AluOpType


def _emit(nc, repeat=1, ablate="full"):
    xh = nc.dram_tensor("xh", [NB, C, LP], FP16, kind="ExternalInput")
    wt = nc.dram_tensor("wt", [ND, CONTRACT, KPD], FP16, kind="ExternalInput")
    bia = nc.dram_tensor("bia", [MT, ND * NKT], F32, kind="ExternalInput")
    out = nc.dram_tensor("out", [NB, 2 * ND * KPD], F32, kind="ExternalOutput")

    for _rep in range(repeat):
        _emit_body(nc, xh, wt, bia, out, ablate)


def _emit_body(nc, xh, wt, bia, out, ablate="full"):
    do_act = ablate in ("full", "pe_act")
    do_dve = ablate in ("full", "pe_dve")
    with tile.TileContext(nc) as tc:
        with (
            tc.tile_pool(name="const", bufs=1) as constp,
            tc.tile_pool(name="xtp", bufs=2) as xtp,
            tc.tile_pool(name="pspa", bufs=(4 if not DUP else 2), space="PSUM") as pspa,
            tc.tile_pool(name="pspb", bufs=2, space="PSUM") as pspb,
            tc.tile_pool(name="pcp", bufs=1, space="PSUM") as pcp,
            tc.tile_pool(name="finp", bufs=1) as finp,
        ):
            lhsT = constp.tile([CONTRACT, ND * KPD], FP16)
            nc.sync.dma_start(
                lhsT.rearrange("r (d m) -> r d m", d=ND),
                wt.ap().rearrange("d r m -> r d m"),
            )
            negb = constp.tile([MT, ND * NKT], F32)
            nc.sync.dma_start(negb[:, :], bia.ap())
            pconst = None
            if PROBE != "none":
                pconst = pcp.tile([MT, 1024], F32)
                nc.vector.memset(pconst[:, :], 1.0)
            trash_v = constp.tile([MT, 1024], FP16)
            trash_a = constp.tile([MT, 1024], FP16)
            slots_mx = [
                constp.tile([MT, ND * NKT * NCH], F32, name=f"smx{b}")
                for b in range(NB)
            ]
            slots_sg = [
                constp.tile([MT, ND * NKT * NCH], F32, name=f"ssg{b}")
                for b in range(NB)
            ]

            for di, d in enumerate(DILS):
                xt = xtp.tile([CONTRACT, NB * L], FP16, tag="xt", name=f"xt{di}")
                for b in range(NB):
                    # One strided DMA per (batch, dilation): rows (j,c) with
                    # tap shift s=(j-4)d read from the zero-padded input, so
                    # every tap is in bounds — no edge-case DMAs.
                    src = bass.AP(
                        xh,
                        b * C * LP + (PAD - 4 * d),
                        [[d, KLEN], [LP, C], [1, L]],
                    )
                    nc.sync.dma_start(xt[:, b * L : b * L + L], src)
                for kt in range(NKT):
                    lhs = lhsT[:, di * KPD + kt * MT : di * KPD + kt * MT + MT]
                    bcol = di * NKT + kt
                    for b in range(NB):
                        for ch, (c0, c1) in enumerate(CHUNKS):
                            w = c1 - c0
                            scol = bcol * NCH + ch
                            # DUP: two independent PSUM copies, A read by
                            # ScalarE, B by VectorE -> no shared-bank
                            # serialization, at the cost of 2x matmuls.
                            # Otherwise one copy read by both (R-R ordered).
                            pa = None
                            if PROBE == "actconst":
                                pass
                            elif do_act or not DUP:
                                pa = pspa.tile(
                                    [MT, 1024], F32, tag="pa",
                                    name=f"pa{di}_{kt}_{b}_{ch}",
                                )
                                for t in range(c0, c1, MM_N):
                                    n = min(MM_N, c1 - t)
                                    nc.tensor.matmul(
                                        pa[:, t - c0 : t - c0 + n],
                                        lhs,
                                        xt[:, b * L + t : b * L + t + n],
                                        start=True,
                                        stop=True,
                                    )
                            if do_act:
                                # ScalarE: PPV via sign(o - b), accumulated sum.
                                # ACT_OUT=psum writes the (unused) sign output
                                # in-place to PSUM (ScE's faster port).
                                pin = pconst if PROBE == "actconst" else pa
                                aout = pin[:, :w] if ACT_OUT == "psum" else trash_a[:, :w]
                                nc.scalar.activation(
                                    aout,
                                    pin[:, :w],
                                    mybir.ActivationFunctionType.Sign,
                                    bias=negb[:, bcol : bcol + 1],
                                    accum_out=slots_sg[b][:, scol : scol + 1],
                                )
                            if do_dve and PROBE == "dveconst":
                                nc.vector.tensor_scalar(
                                    trash_v[:, :w],
                                    pconst[:, :w],
                                    0.0,
                                    None,
                                    op0=ALU.add,
                                    op1=ALU.max,
                                    accum_out=slots_mx[b][:, scol : scol + 1],
                                )
                            elif do_dve:
                                if DUP:
                                    pb = pspb.tile(
                                        [MT, 1024], F32, tag="pb",
                                        name=f"pb{di}_{kt}_{b}_{ch}",
                                    )
                                    for t in range(c0, c1, MM_N):
                                        n = min(MM_N, c1 - t)
                                        nc.tensor.matmul(
                                            pb[:, t - c0 : t - c0 + n],
                                            lhs,
                                            xt[:, b * L + t : b * L + t + n],
                                            start=True,
                                            stop=True,
                                        )
                                else:
                                    pb = pa
                                # VectorE: running max via accum reduce
                                nc.vector.tensor_scalar(
                                    trash_v[:, :w],
                                    pb[:, :w],
                                    0.0,
                                    None,
                                    op0=ALU.add,
                                    op1=ALU.max,
                                    accum_out=slots_mx[b][:, scol : scol + 1],
                                )

            for b in range(NB):
                if not do_act:
                    nc.vector.memset(slots_sg[b][:, :], 0.0)
                if not do_dve:
                    nc.vector.memset(slots_mx[b][:, :], 0.0)

            outv = out.ap().rearrange(
                "bb (d s kt p) -> bb p s d kt", d=ND, s=2, kt=NKT
            )
            for b in range(NB):
                mxr = finp.tile([MT, ND * NKT], F32, name=f"mxr{b}")
                nc.vector.tensor_reduce(
                    mxr[:, :],
                    slots_mx[b].rearrange("p (g c) -> p g c", c=NCH),
                    axis=mybir.AxisListType.X,
                    op=ALU.max,
                )
                sgr = finp.tile([MT, ND * NKT], F32, name=f"sgr{b}")
                nc.vector.tensor_reduce(
                    sgr[:, :],
                    slots_sg[b].rearrange("p (g c) -> p g c", c=NCH),
                    axis=mybir.AxisListType.X,
                    op=ALU.add,
                )
                # ppv = (#gt)/L = (sum_sign + L)/(2L) = sum_sign/(2L) + 0.5
                ppv = finp.tile([MT, ND * NKT], F32, name=f"ppv{b}")
                nc.vector.tensor_scalar(
                    ppv[:, :],
                    sgr[:, :],
                    1.0 / (2.0 * L),
                    0.5,
                    op0=ALU.mult,
                    op1=ALU.add,
                )
                for di in range(ND):
                    nc.sync.dma_start(
                        outv[b, :, 0, di, :], mxr[:, di * NKT : (di + 1) * NKT]
                    )
                    nc.sync.dma_start(
                        outv[b, :, 1, di, :], ppv[:, di * NKT : (di + 1) * NKT]
                    )


_COMPILED = {}


def get_compiled(repeat=1, ablate="full"):
    key = (repeat, ablate)
    if key not in _COMPILED:
        nc = bacc.Bacc(
            "TRN2", target_bir_lowering=False, debug=False, num_devices=NCORES
        )
        _emit(nc, repeat=repeat, ablate=ablate)
        nc.compile()
        _COMPILED[key] = nc
    return _COMPILED[key]


def make_in_maps(x, weights, biases):
    # W[d,k,c,j] -> wt[d, j*12+c, k], matching the Xs row order (j outer, c inner)
    wtr = np.ascontiguousarray(
        weights.astype(np.float16).transpose(0, 3, 2, 1).reshape(ND, CONTRACT, KPD)
    )
    # negated bias, pre-arranged [kernel-in-tile, dilation*ktile] for a
    # contiguous per-partition DMA
    bia = np.ascontiguousarray(
        (-biases.astype(np.float32)).reshape(ND, NKT, MT).transpose(2, 0, 1).reshape(MT, ND * NKT)
    )
    xh = np.zeros((x.shape[0], C, LP), np.float16)
    xh[:, :, PAD : PAD + L] = x.astype(np.float16)
    maps = []
    for c in range(NCORES):
        maps.append(
            {
                "xh": np.ascontiguousarray(xh[NB * c : NB * (c + 1)]),
                "wt": wtr,
                "bia": bia,
            }
        )
    return maps


def run(x, weights, biases, trace=False, **kw):
    nc = get_compiled()
    res = run_bass_kernel_spmd(
        nc, make_in_maps(x, weights, biases), core_ids=list(range(NCORES)),
        trace=trace, **kw
    )
    outs = np.concatenate([r["out"] for r in res.results], axis=0)
    return outs.astype(np.float32), res


def kernel(x, weights, biases):
    out, _ = run(x, weights, biases)
    return out


def bench(x, weights, biases, iters=20, repeat=1, ablate="full"):
    """Time the sharded PJRT executable with pre-staged device inputs.

    Returns (out, per_call_wall_ns_list). Mirrors bass2jax.run_bass_via_pjrt's
    multi-core path, but stages inputs once and times repeated dispatches.
    """
    import time

    import jax
    import jax.numpy as jnp
    from jax.sharding import Mesh, NamedSharding, PartitionSpec
    from jax.experimental.shard_map import shard_map

    import concourse.bass2jax as b2j
    import concourse.mybir as mb

    nc = get_compiled(repeat=repeat, ablate=ablate)
    b2j.install_neuronx_cc_hook()
    in_maps = make_in_maps(x, weights, biases)

    partition_name = nc.partition_id_tensor.name if nc.partition_id_tensor else None
    in_names, out_names, out_avals, zero_outs = [], [], [], []
    for alloc in nc.m.functions[0].allocations:
        if not isinstance(alloc, mb.MemoryLocationSet):
            continue
        name = alloc.memorylocations[0].name
        if alloc.kind == "ExternalInput":
            if name != partition_name:
                in_names.append(name)
        elif alloc.kind == "ExternalOutput":
            out_names.append(name)
            shape = tuple(alloc.tensor_shape)
            dtype = mb.dt.np(alloc.dtype)
            out_avals.append(jax.core.ShapedArray(shape, dtype))
            zero_outs.append(np.zeros(shape, dtype))
    n_params = len(in_names)
    n_outs = len(out_avals)
    all_names = in_names + out_names
    if partition_name is not None:
        all_names = all_names + [partition_name]

    def _body(*args):
        operands = list(args)
        if partition_name is not None:
            operands.append(b2j.partition_id_tensor())
        outs = b2j._bass_exec_p.bind(
            *operands,
            out_avals=tuple(out_avals),
            in_names=tuple(all_names),
            out_names=tuple(out_names),
            lowering_input_output_aliases=(),
            sim_require_finite=True,
            sim_require_nnan=True,
            nc=nc,
        )
        return tuple(outs)

    devices = jax.devices()[:NCORES]
    mesh = Mesh(np.asarray(devices), ("core",))
    spec = PartitionSpec("core")
    sharded = jax.jit(
        shard_map(
            _body,
            mesh=mesh,
            in_specs=(spec,) * (n_params + n_outs),
            out_specs=(spec,) * n_outs,
            check_rep=False,
        ),
        donate_argnums=tuple(range(n_params, n_params + n_outs)),
        keep_unused=True,
    )
    sh = NamedSharding(mesh, spec)
    concat_in = [
        jax.device_put(
            np.concatenate([np.asarray(m[name]) for m in in_maps], axis=0), sh
        )
        for name in in_names
    ]
    zero_host = [np.zeros((NCORES * z.shape[0], *z.shape[1:]), z.dtype) for z in zero_outs]

    times = []
    out_arrs = None
    for i in range(iters + 1):
        zeros_dev = [jax.device_put(z, sh) for z in zero_host]
        jax.block_until_ready(zeros_dev)
        t0 = time.perf_counter()
        out_arrs = sharded(*concat_in, *zeros_dev)
        jax.block_until_ready(out_arrs)
        t1 = time.perf_counter()
        if i > 0:  # skip warmup/compile call
            times.append((t1 - t0) * 1e9)
    out = np.asarray(out_arrs[out_names.index("out")]).reshape(NCORES * NB, -1)
    return out.astype(np.float32), times

